# revision 1
# baseline (speedup 1.0000x reference)
"""PointsFusion Trainium2 kernel.

Pipeline per batch b (B=4, N=4096, k=32):
  knn1 = 32-NN of p1 in p1, knn2 = 32-NN of p1 in p2 (exact, via DVE 8-max rounds)
  gather neighbor coords, features (resi, dist) -> conv(4->64)->BN->relu
  -> conv(64->64)->BN->relu -> conv(64->128)->BN->relu -> channel-max scores
  -> softmax over 64 neighbors -> weighted sum of neighbor coords.

Sharding: 8 cores = (batch b, half h of the 4096 query points). BatchNorm uses
global batch stats -> 3 tiny AllReduces of per-channel sum/sumsq.

Layouts (per 128-query tile):
  pixel space: 16 chunks of 512; chunk c = kn*8+g, pixel j = c*512 + s*16 + p
  (g = query group, p = query-in-group, s = neighbor slot, kn = which knn).
  64-channel activations are packed [128, 4096]: chunk c lives at partitions
  64*(c%2)..+64, free 512*(c//2)..+512 (keeps matmul rhs bases in {0, 64}).

Self-contained: hardcodes shapes; no sibling imports.
"""

import sys

import numpy as np

for _p in ("/opt/trn_rl_repo", "/opt/pypackages"):
    if _p not in sys.path:
        sys.path.append(_p)

import concourse.bass as bass  # noqa: E402  (imported for side effects/typing)
import concourse.mybir as mybir  # noqa: E402
import concourse.tile as tile  # noqa: E402
from concourse import bacc, bass_isa  # noqa: E402
from concourse.bass_utils import run_bass_kernel_spmd  # noqa: E402
from concourse.masks import make_identity  # noqa: E402

F32 = mybir.dt.float32
F32R = mybir.dt.float32r
U16 = mybir.dt.uint16
I16 = mybir.dt.int16
AF = mybir.ActivationFunctionType
OP = mybir.AluOpType

NCORES = 8
B = 4
N = 4096          # candidate points per batch
KNN = 32          # neighbors per knn
QPC = 2048        # query points per core
NT = 16           # query tiles of 128 per core
C1, C2, C3 = 64, 64, 128
NTOT = float(B * N * 2 * KNN)   # BN stat count (global)
BN_EPS = 1e-3
NEG = -1.0e30


def _pk(cc):
    """packed [128, 4096] slice coords for chunk cc."""
    return 64 * (cc % 2), 512 * (cc // 2)


def _build_program(single=False):
    nc = bacc.Bacc(
        "TRN2", target_bir_lowering=False, debug=False,
        num_devices=1 if single else NCORES,
    )
    nc._single_core_nocoll = single

    ap = {}
    def din(name, shape):
        ap[name] = nc.dram_tensor(name, shape, F32, kind="ExternalInput").ap()
    din("qf", [4, QPC])
    din("t1", [4, N])
    din("t2", [4, N])
    din("gt", [128, N])
    din("qr", [4, QPC])
    din("qsq", [128, NT])
    din("w1t", [4, C1])
    din("w2t", [128, C2])     # duplicated at partition 64
    din("w3t", [128, C3])     # duplicated at partition 64
    din("gt2", [128, N])
    din("gb1", [C1, 2])
    din("gb2", [C2, 2])
    din("gb3", [C3, 2])
    din("selw", [8, 128])

    ap["out"] = nc.dram_tensor("out", [3, QPC], F32, kind="ExternalOutput").ap()

    ap["y1d"] = nc.dram_tensor("y1d", [NT, 128, 4096], F32).ap()
    ap["y2d"] = nc.dram_tensor("y2d", [NT, 128, 4096], F32).ap()
    ap["y3d"] = nc.dram_tensor("y3d", [NT, C3, 8192], F32).ap()
    ap["g1d"] = nc.dram_tensor("g1d", [NT, 128, 512], F32).ap()
    ap["g2d"] = nc.dram_tensor("g2d", [NT, 128, 512], F32).ap()
    ap["dsd"] = nc.dram_tensor("dsd", [NT, 8192], F32).ap()
    for i, c in ((0, C1), (1, C2), (2, C3)):
        ap[f"arin{i}"] = nc.dram_tensor(f"arin{i}", [c * 2], F32).ap()
        ap[f"arout{i}"] = nc.dram_tensor(f"arout{i}", [c * 2], F32).ap()

    with tile.TileContext(nc) as tc:
        _kernel_body(tc, ap)
    nc.compile()
    return nc


def _kernel_body(tc, d):
    nc = tc.nc
    from contextlib import ExitStack

    ctx = ExitStack()
    with ctx:
        cpool = ctx.enter_context(tc.tile_pool(name="consts", bufs=1))
        t1 = cpool.tile([4, N], F32)
        t2 = cpool.tile([4, N], F32)
        gt = cpool.tile([128, N], F32)
        qf = cpool.tile([4, QPC], F32)
        qr = cpool.tile([4, QPC], F32)
        qsq = cpool.tile([128, NT], F32)
        w1 = cpool.tile([4, C1], F32)
        w2 = cpool.tile([128, C2], F32)
        w3 = cpool.tile([128, C3], F32)
        gb1 = cpool.tile([C1, 2], F32)
        gb2 = cpool.tile([C2, 2], F32)
        gb3 = cpool.tile([C3, 2], F32)
        gt2 = cpool.tile([128, N], F32)
        selw = cpool.tile([8, 128], F32)
        ident = cpool.tile([128, 128], F32)
        make_identity(nc, ident[:])
        for nm, sb in [("t1", t1), ("t2", t2), ("gt", gt), ("gt2", gt2),
                       ("qf", qf),
                       ("qr", qr), ("qsq", qsq), ("w1t", w1), ("w2t", w2),
                       ("w3t", w3), ("gb1", gb1), ("gb2", gb2), ("gb3", gb3),
                       ("selw", selw)]:
            nc.sync.dma_start(out=sb[:], in_=d[nm][:])

        spool = ctx.enter_context(tc.tile_pool(name="stats", bufs=1))
        sm1 = spool.tile([C1, NT * 16], F32)
        sq1 = spool.tile([C1, NT * 16], F32)
        sm2 = spool.tile([C2, NT * 16], F32)
        sq2 = spool.tile([C2, NT * 16], F32)
        sm3 = spool.tile([C3, NT * 16], F32)
        sq3 = spool.tile([C3, NT * 16], F32)
        ab1 = spool.tile([128, 2], F32)   # col0 = scale a, col1 = bias b (dup at 64)
        ab2 = spool.tile([128, 2], F32)
        ab3 = spool.tile([C3, 2], F32)

        # ---------------- Phase 1: knn + gather + feat + conv1 ----------------
        with tc.tile_pool(name="p1m", bufs=2) as mpool, \
             tc.tile_pool(name="p1psum", bufs=2, space="PSUM") as pp, \
             tc.tile_pool(name="p1tp", bufs=2, space="PSUM") as tpp, \
             tc.tile_pool(name="p1cpsum", bufs=3, space="PSUM") as cp, \
             tc.tile_pool(name="p1feat", bufs=1) as fpool, \
             tc.tile_pool(name="p1work", bufs=2) as wp, \
             tc.tile_pool(name="p1y", bufs=2) as yp:
            for t in range(NT):
                msb = mpool.tile([128, N], F32, tag="msb")
                vals = wp.tile([128, 64], F32, tag="vals")
                idxu = wp.tile([128, 64], U16, tag="idxu")
                idxi = wp.tile([128, 64], I16, tag="idxi")
                for kn, tab in ((0, t1), (1, t2)):
                    # M = 2 q.c - |c|^2 (maximize == nearest)
                    for ch in range(8):
                        pm = pp.tile([128, 512], F32, tag="pm")
                        nc.tensor.matmul(
                            out=pm[:],
                            lhsT=qf[:, t * 128:(t + 1) * 128],
                            rhs=tab[:, ch * 512:(ch + 1) * 512],
                            start=True, stop=True,
                        )
                        nc.scalar.activation(
                            out=msb[:, ch * 512:(ch + 1) * 512], in_=pm[:],
                            func=AF.Identity)
                    # top-32 rounds
                    for r in range(4):
                        v8 = vals[:, kn * 32 + r * 8: kn * 32 + r * 8 + 8]
                        i8 = idxu[:, kn * 32 + r * 8: kn * 32 + r * 8 + 8]
                        nc.vector.max(out=v8, in_=msb[:])
                        nc.vector.max_index(out=i8, in_max=v8, in_values=msb[:])
                        if r < 3:
                            nc.vector.match_replace(
                                out=msb[:], in_to_replace=v8,
                                in_values=msb[:], imm_value=NEG)
                nc.vector.tensor_copy(out=idxi[:], in_=idxu[:])

                # gather neighbor coords; both tables carry xyz on band rows
                # 16g+{0..2} (gt = p1 for knn1, gt2 = p2 for knn2); spill raw
                # for the fusion phase
                g1 = wp.tile([128, 512], F32, tag="g1")
                g2 = wp.tile([128, 512], F32, tag="g2")
                nc.gpsimd.ap_gather(
                    out_ap=g1[:], in_ap=gt[:], idxs_ap=idxi[:, 0:32],
                    channels=128, num_elems=N, d=1, num_idxs=512)
                nc.gpsimd.ap_gather(
                    out_ap=g2[:], in_ap=gt2[:], idxs_ap=idxi[:, 32:64],
                    channels=128, num_elems=N, d=1, num_idxs=512)
                nc.sync.dma_start(out=d["g1d"][t], in_=g1[:])
                nc.sync.dma_start(out=d["g2d"][t], in_=g2[:])

                # conv1 rhs must start at partition 0: DMA bands into a flat
                # [4, 8192] tile
                feat = fpool.tile([4, 8192], F32, tag="feat")
                for g in range(8):
                    nc.sync.dma_start(
                        out=feat[0:3, g * 512:(g + 1) * 512],
                        in_=g1[16 * g: 16 * g + 3, :])
                    nc.sync.dma_start(
                        out=feat[0:3, (8 + g) * 512:(9 + g) * 512],
                        in_=g2[16 * g: 16 * g + 3, :])

                # dist = sqrt(max(|q|^2 - val, 0)) into feat row 3
                d2 = wp.tile([128, 64], F32, tag="d2")
                nc.vector.tensor_scalar(
                    out=d2[:], in0=vals[:], scalar1=qsq[:, t:t + 1],
                    scalar2=-1.0, op0=OP.subtract, op1=OP.mult)
                nc.vector.tensor_scalar_max(d2[:], d2[:], 0.0)
                nc.scalar.activation(out=d2[:], in_=d2[:], func=AF.Sqrt)
                # shuffle dist to pixel layout: PE-transpose to [nbr, query],
                # then per-chunk DMAs with contiguous 16-wide runs
                dtp = tpp.tile([64, 128], F32, tag="dtp")
                nc.tensor.transpose(out=dtp[:], in_=d2[:], identity=ident[:])
                d2t = wp.tile([64, 128], F32, tag="d2t")
                nc.scalar.activation(out=d2t[:], in_=dtp[:], func=AF.Identity)
                for kn in (0, 1):
                    for g in range(8):
                        c = kn * 8 + g
                        nc.sync.dma_start(
                            out=feat[3:4, c * 512:(c + 1) * 512]
                                .rearrange("c (s p) -> c s p", s=32),
                            in_=d2t[kn * 32:(kn + 1) * 32,
                                    16 * g:16 * g + 16])

                # resi = nn - q (in place on coord rows)
                qrt = qr[0:3, t * 128:(t + 1) * 128]
                for kn in (0, 1):
                    nc.vector.tensor_tensor(
                        out=feat[0:3, kn * 4096:(kn + 1) * 4096]
                            .rearrange("c (g s p) -> c g s p", g=8, s=32),
                        in0=feat[0:3, kn * 4096:(kn + 1) * 4096]
                            .rearrange("c (g s p) -> c g s p", g=8, s=32),
                        in1=qrt.rearrange("c (g p) -> c g p", g=8)
                            .unsqueeze(2).to_broadcast([3, 8, 32, 16]),
                        op=OP.subtract)

                # conv1: 16 chunks -> y1 packed [128, 4096]
                y1 = yp.tile([128, 4096], F32, tag="y1")
                for c in range(16):
                    bp_, fo = _pk(c)
                    pc = cp.tile([C1, 512], F32, tag="pc1")
                    nc.tensor.matmul(
                        out=pc[:],
                        lhsT=w1[:],
                        rhs=feat[:, c * 512:(c + 1) * 512],
                        start=True, stop=True)
                    nc.scalar.activation(
                        out=y1[bp_:bp_ + 64, fo:fo + 512], in_=pc[:],
                        func=AF.Identity,
                        accum_out=sm1[:, t * 16 + c: t * 16 + c + 1])
                    sqs = wp.tile([C1, 512], F32, tag="sqs")
                    nc.scalar.activation(
                        out=sqs[:], in_=pc[:], func=AF.Square,
                        accum_out=sq1[:, t * 16 + c: t * 16 + c + 1])
                nc.sync.dma_start(out=d["y1d"][t], in_=y1[:])

        _bn_allreduce(tc, 0, sm1, sq1, gb1, ab1, d["arin0"], d["arout0"], True)

        # ---------------- Phase 2: apply BN1+relu, conv2 ----------------
        with tc.tile_pool(name="p2y", bufs=2) as yp, \
             tc.tile_pool(name="p2psum", bufs=4, space="PSUM") as cp, \
             tc.tile_pool(name="p2work", bufs=2) as wp:
            for t in range(NT):
                y1 = yp.tile([128, 4096], F32, tag="y1l")
                nc.sync.dma_start(out=y1[:], in_=d["y1d"][t])
                nc.scalar.activation(
                    out=y1[:], in_=y1[:], func=AF.Relu,
                    scale=ab1[:, 0:1], bias=ab1[:, 1:2])
                y2 = yp.tile([128, 4096], F32, tag="y2")
                for c in range(16):
                    bp_, fo = _pk(c)
                    pc = cp.tile([C2, 512], F32, tag="pc2")
                    nc.tensor.matmul(
                        out=pc[:], lhsT=w2[bp_:bp_ + 64, :],
                        rhs=y1[bp_:bp_ + 64, fo:fo + 512],
                        start=True, stop=True)
                    nc.scalar.activation(
                        out=y2[bp_:bp_ + 64, fo:fo + 512], in_=pc[:],
                        func=AF.Identity,
                        accum_out=sm2[:, t * 16 + c: t * 16 + c + 1])
                    sqs = wp.tile([C2, 512], F32, tag="sqs2")
                    nc.scalar.activation(
                        out=sqs[:], in_=pc[:], func=AF.Square,
                        accum_out=sq2[:, t * 16 + c: t * 16 + c + 1])
                nc.sync.dma_start(out=d["y2d"][t], in_=y2[:])

        _bn_allreduce(tc, 1, sm2, sq2, gb2, ab2, d["arin1"], d["arout1"], True)

        # ---------------- Phase 3: apply BN2+relu, conv3 ----------------
        with tc.tile_pool(name="p3y", bufs=2) as yp, \
             tc.tile_pool(name="p3psum", bufs=4, space="PSUM") as cp, \
             tc.tile_pool(name="p3work", bufs=2) as wp:
            for t in range(NT):
                y2 = yp.tile([128, 4096], F32, tag="y2l")
                nc.sync.dma_start(out=y2[:], in_=d["y2d"][t])
                nc.scalar.activation(
                    out=y2[:], in_=y2[:], func=AF.Relu,
                    scale=ab2[:, 0:1], bias=ab2[:, 1:2])
                y3 = yp.tile([C3, 8192], F32, tag="y3")
                for c in range(16):
                    bp_, fo = _pk(c)
                    pc = cp.tile([C3, 512], F32, tag="pc3")
                    nc.tensor.matmul(
                        out=pc[:], lhsT=w3[bp_:bp_ + 64, :],
                        rhs=y2[bp_:bp_ + 64, fo:fo + 512],
                        start=True, stop=True)
                    nc.scalar.activation(
                        out=y3[:, c * 512:(c + 1) * 512], in_=pc[:],
                        func=AF.Identity,
                        accum_out=sm3[:, t * 16 + c: t * 16 + c + 1])
                    sqs = wp.tile([C3, 512], F32, tag="sqs3")
                    nc.scalar.activation(
                        out=sqs[:], in_=pc[:], func=AF.Square,
                        accum_out=sq3[:, t * 16 + c: t * 16 + c + 1])
                nc.sync.dma_start(out=d["y3d"][t], in_=y3[:])

        _bn_allreduce(tc, 2, sm3, sq3, gb3, ab3, d["arin2"], d["arout2"], False)

        # ------------- Phase 4: scores, softmax, fusion, output -------------
        with tc.tile_pool(name="p4y", bufs=2) as yp, \
             tc.tile_pool(name="p4work", bufs=2) as wp, \
             tc.tile_pool(name="p4psum", bufs=2, space="PSUM") as pp4, \
             tc.tile_pool(name="p4out", bufs=1) as op_:
            outsb = op_.tile([4, QPC], F32)
            for t in range(NT):
                y3 = yp.tile([C3, 8192], F32, tag="y3l")
                nc.sync.dma_start(out=y3[:], in_=d["y3d"][t])
                nc.scalar.activation(
                    out=y3[:], in_=y3[:], func=AF.Relu,
                    scale=ab3[:, 0:1], bias=ab3[:, 1:2])
                # channel-max scores, split by knn half (engine partition
                # bases must be 32-aligned, so rows land via DMA)
                scA = wp.tile([8, 512], F32, tag="scA")
                scB = wp.tile([8, 512], F32, tag="scB")
                par = wp.tile([128, 512], F32, tag="par")
                for c in range(16):
                    nc.gpsimd.partition_all_reduce(
                        out_ap=par[:], in_ap=y3[:, c * 512:(c + 1) * 512],
                        channels=128, reduce_op=bass_isa.ReduceOp.max)
                    dst = scA if c < 8 else scB
                    nc.sync.dma_start(out=dst[c % 8: c % 8 + 1, :],
                                      in_=par[0:1, :])
                # softmax over the 64 neighbors of each query
                qmA = wp.tile([8, 16], F32, tag="qmA")
                qmB = wp.tile([8, 16], F32, tag="qmB")
                for sct, qm in ((scA, qmA), (scB, qmB)):
                    nc.vector.tensor_reduce(
                        out=qm[:],
                        in_=sct[:].rearrange("c (s p) -> c p s", s=32),
                        axis=mybir.AxisListType.X, op=OP.max)
                nc.vector.tensor_tensor(
                    out=qmA[:], in0=qmA[:], in1=qmB[:], op=OP.max)
                exA = wp.tile([8, 512], F32, tag="exA")
                exB = wp.tile([8, 512], F32, tag="exB")
                for sct, ext in ((scA, exA), (scB, exB)):
                    nc.vector.tensor_tensor(
                        out=ext[:].rearrange("c (s p) -> c s p", s=32),
                        in0=sct[:].rearrange("c (s p) -> c s p", s=32),
                        in1=qmA[:].unsqueeze(1).to_broadcast([8, 32, 16]),
                        op=OP.subtract)
                    nc.scalar.activation(out=ext[:], in_=ext[:], func=AF.Exp)
                esA = wp.tile([8, 16], F32, tag="esA")
                esB = wp.tile([8, 16], F32, tag="esB")
                for ext, est in ((exA, esA), (exB, esB)):
                    nc.vector.tensor_reduce(
                        out=est[:],
                        in_=ext[:].rearrange("c (s p) -> c p s", s=32),
                        axis=mybir.AxisListType.X, op=OP.add)
                nc.vector.tensor_tensor(
                    out=esA[:], in0=esA[:], in1=esB[:], op=OP.add)
                nc.vector.reciprocal(out=esA[:], in_=esA[:])
                for ext in (exA, exB):
                    nc.vector.tensor_tensor(
                        out=ext[:].rearrange("c (s p) -> c s p", s=32),
                        in0=ext[:].rearrange("c (s p) -> c s p", s=32),
                        in1=esA[:].unsqueeze(1).to_broadcast([8, 32, 16]),
                        op=OP.mult)
                # fusion: replicate weight rows onto band partitions via a
                # selector matmul, multiply with raw coords, segment-reduce
                g1 = wp.tile([128, 512], F32, tag="g1l")
                g2 = wp.tile([128, 512], F32, tag="g2l")
                nc.sync.dma_start(out=g1[:], in_=d["g1d"][t])
                nc.sync.dma_start(out=g2[:], in_=d["g2d"][t])
                wr1 = wp.tile([128, 512], F32, tag="wr1")
                wr2 = wp.tile([128, 512], F32, tag="wr2")
                for ext, wr in ((exA, wr1), (exB, wr2)):
                    pw = pp4.tile([128, 512], F32, tag="pw")
                    nc.tensor.matmul(
                        out=pw[:], lhsT=selw[:],
                        rhs=ext[:], start=True, stop=True)
                    nc.scalar.activation(out=wr[:], in_=pw[:], func=AF.Identity)
                pr = wp.tile([128, 512], F32, tag="pr")
                nc.vector.tensor_tensor(out=pr[:], in0=g1[:], in1=wr1[:],
                                        op=OP.mult)
                nc.vector.tensor_tensor(out=wr2[:], in0=g2[:], in1=wr2[:],
                                        op=OP.mult)
                nc.vector.tensor_tensor(out=pr[:], in0=pr[:], in1=wr2[:],
                                        op=OP.add)
                fp = wp.tile([128, 16], F32, tag="fp")
                nc.vector.tensor_reduce(
                    out=fp[:], in_=pr[:].rearrange("c (s p) -> c p s", s=32),
                    axis=mybir.AxisListType.X, op=OP.add)
                for g in range(8):
                    nc.sync.dma_start(
                        out=outsb[0:3,
                                  t * 128 + 16 * g: t * 128 + 16 * g + 16],
                        in_=fp[16 * g: 16 * g + 3, :])
            nc.sync.dma_start(out=d["out"][:], in_=outsb[0:3, :])


def _bn_allreduce(tc, li, sm, sq, gbe, ab, arin, arout, dup):
    """Reduce per-chunk stat slots, AllReduce across 8 cores, compute
    per-channel scale a = g*rsqrt(var+eps) and bias b = be - a*mean."""
    nc = tc.nc
    C = sm.shape[0]
    with tc.tile_pool(name=f"bn{li}", bufs=1) as bp:
        st = bp.tile([C, 2], F32)
        nc.vector.tensor_reduce(out=st[:, 0:1], in_=sm[:],
                                axis=mybir.AxisListType.X, op=OP.add)
        nc.vector.tensor_reduce(out=st[:, 1:2], in_=sq[:],
                                axis=mybir.AxisListType.X, op=OP.add)
        nc.sync.dma_start(out=arin[:], in_=st[:])
        if getattr(nc, "_single_core_nocoll", False):
            nc.sync.dma_start(out=arout[:], in_=arin[:])
        else:
            nc.gpsimd.collective_compute(
                "AllReduce", OP.add, replica_groups=[list(range(NCORES))],
                ins=[arin.opt()], outs=[arout.opt()])
        ar = bp.tile([C, 2], F32)
        nc.sync.dma_start(out=ar[:], in_=arout[:])
        mean = bp.tile([C, 1], F32)
        var = bp.tile([C, 1], F32)
        nc.vector.tensor_scalar_mul(mean[:], ar[:, 0:1], 1.0 / NTOT)
        nc.vector.tensor_scalar_mul(var[:], ar[:, 1:2], 1.0 / NTOT)
        m2 = bp.tile([C, 1], F32)
        nc.vector.tensor_tensor(out=m2[:], in0=mean[:], in1=mean[:], op=OP.mult)
        nc.vector.tensor_tensor(out=var[:], in0=var[:], in1=m2[:], op=OP.subtract)
        nc.vector.tensor_scalar_add(var[:], var[:], BN_EPS)
        nc.scalar.activation(out=var[:], in_=var[:], func=AF.Sqrt)
        nc.vector.reciprocal(out=var[:], in_=var[:])  # rsqrt(var+eps)
        nc.vector.tensor_tensor(out=ab[0:C, 0:1], in0=var[:], in1=gbe[:, 0:1],
                                op=OP.mult)            # a
        nc.vector.tensor_tensor(out=m2[:], in0=ab[0:C, 0:1], in1=mean[:],
                                op=OP.mult)
        nc.vector.tensor_tensor(out=ab[0:C, 1:2], in0=gbe[:, 1:2], in1=m2[:],
                                op=OP.subtract)        # b = be - a*mean
        if dup:
            nc.vector.tensor_copy(out=ab[C:2 * C, :], in_=ab[0:C, :])


_PROGRAM = None
LAST_RESULT = None


def _get_program():
    global _PROGRAM
    if _PROGRAM is None:
        _PROGRAM = _build_program()
    return _PROGRAM


def _prep_core_inputs(points1, points2, W1, W2, W3, gs, bes, b, h):
    p1 = points1[b]          # [3, N]
    p2 = points2[b]
    q = p1[:, h * QPC:(h + 1) * QPC]            # [3, QPC]
    qf = np.concatenate([2.0 * q, np.ones((1, QPC), np.float32)], axis=0)

    def cand_tab(p):
        sq = (p * p).sum(axis=0, keepdims=True)
        return np.concatenate([p, -sq], axis=0).astype(np.float32)  # [4, N]

    gtab = np.zeros((128, N), np.float32)
    gtab2 = np.zeros((128, N), np.float32)
    for g in range(8):
        gtab[16 * g + 0:16 * g + 3] = p1
        gtab2[16 * g + 0:16 * g + 3] = p2
    qraw = np.zeros((4, QPC), np.float32)
    qraw[0:3] = q
    qsqv = (q * q).sum(axis=0).reshape(NT, 128).T.astype(np.float32)  # [128, NT]

    def dup128(w):      # [64, C] -> [128, C] duplicated
        return np.concatenate([w, w], axis=0).astype(np.float32)

    selw = np.zeros((8, 128), np.float32)
    for g in range(8):
        for c3 in range(3):
            selw[g, 16 * g + c3] = 1.0

    return {
        "selw": selw,
        "qf": qf.astype(np.float32),
        "t1": cand_tab(p1), "t2": cand_tab(p2), "gt": gtab, "gt2": gtab2,
        "qr": qraw, "qsq": np.ascontiguousarray(qsqv),
        "w1t": np.ascontiguousarray(W1.T).astype(np.float32),
        "w2t": dup128(np.ascontiguousarray(W2.T)),
        "w3t": dup128(np.ascontiguousarray(W3.T)),
        "gb1": np.stack([gs[0], bes[0]], axis=1).astype(np.float32),
        "gb2": np.stack([gs[1], bes[1]], axis=1).astype(np.float32),
        "gb3": np.stack([gs[2], bes[2]], axis=1).astype(np.float32),
    }


def kernel(points1, points2, k, t, W1, b1, g1, be1, W2, b2, g2, be2,
           W3, b3, g3, be3):
    # b1/b2/b3 cancel inside train-mode BatchNorm; t is unused by the net.
    assert int(np.asarray(k)) == KNN
    points1 = np.asarray(points1, np.float32)
    points2 = np.asarray(points2, np.float32)
    gs = [np.asarray(g1, np.float32), np.asarray(g2, np.float32),
          np.asarray(g3, np.float32)]
    bes = [np.asarray(be1, np.float32), np.asarray(be2, np.float32),
           np.asarray(be3, np.float32)]
    Ws = [np.asarray(W1, np.float32), np.asarray(W2, np.float32),
          np.asarray(W3, np.float32)]

    in_maps = []
    for c in range(NCORES):
        b, h = divmod(c, 2)
        in_maps.append(_prep_core_inputs(points1, points2, *Ws, gs, bes, b, h))

    nc = _get_program()
    bkr = run_bass_kernel_spmd(nc, in_maps, list(range(NCORES)))
    global LAST_RESULT
    LAST_RESULT = bkr
    res = bkr.results

    out = np.zeros((B, 3, N), np.float32)
    for c in range(NCORES):
        b, h = divmod(c, 2)
        out[b, :, h * QPC:(h + 1) * QPC] = res[c]["out"]
    return out



# revision 41
# speedup vs baseline: 1.1614x; 1.1614x over previous
"""PointsFusion Trainium2 kernel.

Pipeline per batch b (B=4, N=4096, k=32):
  knn1 = 32-NN of p1 in p1, knn2 = 32-NN of p1 in p2 (exact, via DVE 8-max rounds)
  gather neighbor coords, features (resi, dist) -> conv(4->64)->BN->relu
  -> conv(64->64)->BN->relu -> conv(64->128)->BN->relu -> channel-max scores
  -> softmax over 64 neighbors -> weighted sum of neighbor coords.

Sharding: 8 cores = (batch b, half h of the 4096 query points). BatchNorm uses
global batch stats -> 3 tiny AllReduces of per-channel sum/sumsq.

Layouts (per 128-query tile):
  pixel space: 16 chunks of 512; chunk c = kn*8+g, pixel j = c*512 + s*16 + p
  (g = query group, p = query-in-group, s = neighbor slot, kn = which knn).
  64-channel activations are packed [128, 4096]: chunk c lives at partitions
  64*(c%2)..+64, free 512*(c//2)..+512 (keeps matmul rhs bases in {0, 64}).

Self-contained: hardcodes shapes; no sibling imports.
"""

import sys

import numpy as np

for _p in ("/opt/trn_rl_repo", "/opt/pypackages"):
    if _p not in sys.path:
        sys.path.append(_p)

import concourse.bass as bass  # noqa: E402  (imported for side effects/typing)  # noqa: F401
import concourse.mybir as mybir  # noqa: E402
import concourse.tile as tile  # noqa: E402
from concourse import bacc, bass_isa  # noqa: E402
from concourse.bass_utils import run_bass_kernel_spmd  # noqa: E402
from concourse.masks import make_identity  # noqa: E402

F32 = mybir.dt.float32
F32R = mybir.dt.float32r
U16 = mybir.dt.uint16
I16 = mybir.dt.int16
AF = mybir.ActivationFunctionType
OP = mybir.AluOpType


USE_F32R = True


def _r(ap):
    """view an f32 AP as f32r for full-rate PE streaming"""
    return ap.bitcast(F32R) if USE_F32R else ap

NCORES = 8
B = 4
N = 4096          # candidate points per batch
KNN = 32          # neighbors per knn
QPC = 2048        # query points per core
NT = 16           # query tiles of 128 per core
C1, C2, C3 = 64, 64, 128
NTOT = float(B * N * 2 * KNN)   # BN stat count (global)
BN_EPS = 1e-3
NEG = -1.0e30


def _pk(cc):
    """packed [128, 4096] slice coords for chunk cc (chunks 0-7 on the lower
    partition half, 8-15 on the upper; matmul PSUM outs stay at base 0)."""
    return 64 * (cc // 8), 512 * (cc % 8)


def _build_program(single=False):
    nc = bacc.Bacc(
        "TRN2", target_bir_lowering=False, debug=False,
        num_devices=1 if single else NCORES,
    )
    nc._single_core_nocoll = single

    ap = {}
    def din(name, shape):
        ap[name] = nc.dram_tensor(name, shape, F32, kind="ExternalInput").ap()
    din("qf", [4, QPC])
    din("t1", [4, N])
    din("t2", [4, N])
    din("gt", [128, N])
    din("qr", [4, QPC])
    din("qb", [128, NT])
    din("w1t", [4, C1])
    din("w2t", [128, C2])     # duplicated at partition 64
    din("w3t", [128, C3])     # duplicated at partition 64
    din("gt2", [128, N])
    din("gb1", [C1, 2])
    din("gb2", [C2, 2])
    din("gb3", [C3, 2])
    din("selw", [8, 128])

    ap["out"] = nc.dram_tensor("out", [3, QPC], F32, kind="ExternalOutput").ap()

    ap["y1d"] = nc.dram_tensor("y1d", [NT, 128, 4096], F32).ap()
    ap["y2d"] = nc.dram_tensor("y2d", [NT, 128, 4096], F32).ap()
    ap["y3d"] = nc.dram_tensor("y3d", [NT, C3, 8192], F32).ap()
    ap["g1d"] = nc.dram_tensor("g1d", [NT, 128, 512], F32).ap()
    ap["g2d"] = nc.dram_tensor("g2d", [NT, 128, 512], F32).ap()
    ap["dsd"] = nc.dram_tensor("dsd", [NT, 8192], F32).ap()
    for i, c in ((0, C1), (1, C2), (2, C3)):
        ap[f"arin{i}"] = nc.dram_tensor(f"arin{i}", [c * 2], F32).ap()
        ap[f"arout{i}"] = nc.dram_tensor(f"arout{i}", [c * 2], F32).ap()

    with tile.TileContext(nc) as tc:
        _kernel_body(tc, ap)
    nc.compile()
    return nc


def _kernel_body(tc, d):
    nc = tc.nc
    from contextlib import ExitStack

    ctx = ExitStack()
    with ctx:
        cpool = ctx.enter_context(tc.tile_pool(name="consts", bufs=1))
        t1 = cpool.tile([4, N], F32)
        t2 = cpool.tile([4, N], F32)
        gt = cpool.tile([128, N], F32)
        qf = cpool.tile([4, QPC], F32)
        qr = cpool.tile([4, QPC], F32)
        qb = cpool.tile([128, NT], F32)
        w1 = cpool.tile([4, C1], F32)
        w2 = cpool.tile([128, C2], F32)
        w3 = cpool.tile([128, C3], F32)
        gb1 = cpool.tile([C1, 2], F32)
        gb2 = cpool.tile([C2, 2], F32)
        gb3 = cpool.tile([C3, 2], F32)
        gt2 = cpool.tile([128, N], F32)
        selw = cpool.tile([8, 128], F32)
        ident = cpool.tile([128, 128], F32)
        make_identity(nc, ident[:])
        # tiles consumed by fp32r matmuls get f32r-typed producer DMAs so the
        # BIR verifier sees a consistently-rounded chain
        r_consts = {"w1t", "w2t", "w3t", "selw"}
        for nm, sb in [("t1", t1), ("t2", t2), ("gt", gt), ("gt2", gt2),
                       ("qf", qf),
                       ("qr", qr), ("qb", qb), ("w1t", w1), ("w2t", w2),
                       ("w3t", w3), ("gb1", gb1), ("gb2", gb2), ("gb3", gb3),
                       ("selw", selw)]:
            if nm in r_consts:
                nc.sync.dma_start(out=_r(sb[:]), in_=_r(d[nm][:]))
            else:
                nc.sync.dma_start(out=sb[:], in_=d[nm][:])

        spool = ctx.enter_context(tc.tile_pool(name="stats", bufs=1))
        sm1 = spool.tile([C1, NT * 4], F32)
        sq1 = spool.tile([C1, NT * 4], F32)
        sm2 = spool.tile([C2, NT * 4], F32)
        sq2 = spool.tile([C2, NT * 4], F32)
        sm3 = spool.tile([C3, NT * 4], F32)
        sq3 = spool.tile([C3, NT * 4], F32)
        ab1 = spool.tile([128, 2], F32)   # col0 = scale a, col1 = bias b (dup at 64)
        ab2 = spool.tile([128, 2], F32)
        ab3 = spool.tile([C3, 2], F32)

        # ---------------- Phase 1: knn + gather + feat + conv1 ----------------
        with tc.tile_pool(name="p1m", bufs=2) as mpool, \
             tc.tile_pool(name="p1psum", bufs=1, space="PSUM") as pp, \
             tc.tile_pool(name="p1cpsum", bufs=1, space="PSUM") as cp, \
             tc.tile_pool(name="p1feat", bufs=1) as fpool, \
             tc.tile_pool(name="p1work", bufs=2) as wp, \
             tc.tile_pool(name="p1y", bufs=2) as yp:
            for t in range(NT):
                msb = mpool.tile([128, N], F32, tag="msb")
                vals = wp.tile([128, 64], F32, tag="vals")
                idxu = wp.tile([128, 64], U16, tag="idxu")
                idxi = wp.tile([128, 64], I16, tag="idxi")
                for kn, tab in ((0, t1), (1, t2)):
                    # M = 2 q.c - |c|^2 (maximize == nearest); fp32 matmul —
                    # fp32r's extra rounding flips near-boundary neighbors
                    for ch in range(2):
                        pm = pp.tile([128, 2048], F32, tag="pm")
                        for h in range(4):
                            nc.tensor.matmul(
                                out=pm[:, h * 512:(h + 1) * 512],
                                lhsT=qf[:, t * 128:(t + 1) * 128],
                                rhs=tab[:, (4 * ch + h) * 512:
                                        (4 * ch + h + 1) * 512],
                                start=True, stop=True,
                            )
                        nc.scalar.activation(
                            out=msb[:, ch * 2048:(ch + 1) * 2048], in_=pm[:],
                            func=AF.Identity)
                    # top-32 rounds
                    for r in range(4):
                        v8 = vals[:, kn * 32 + r * 8: kn * 32 + r * 8 + 8]
                        i8 = idxu[:, kn * 32 + r * 8: kn * 32 + r * 8 + 8]
                        nc.vector.max(out=v8, in_=msb[:])
                        nc.vector.max_index(out=i8, in_max=v8, in_values=msb[:])
                        if r < 3:
                            nc.vector.match_replace(
                                out=msb[:], in_to_replace=v8,
                                in_values=msb[:], imm_value=NEG)
                nc.vector.tensor_copy(out=idxi[:], in_=idxu[:])

                # gather neighbor coords; both tables carry xyz on band rows
                # 16g+{0..2} (gt = p1 for knn1, gt2 = p2 for knn2); spill raw
                # for the fusion phase
                g1 = wp.tile([128, 512], F32, tag="g1")
                g2 = wp.tile([128, 512], F32, tag="g2")
                nc.gpsimd.ap_gather(
                    out_ap=g1[:], in_ap=gt[:], idxs_ap=idxi[:, 0:32],
                    channels=128, num_elems=N, d=1, num_idxs=512)
                nc.gpsimd.ap_gather(
                    out_ap=g2[:], in_ap=gt2[:], idxs_ap=idxi[:, 32:64],
                    channels=128, num_elems=N, d=1, num_idxs=512)
                nc.sync.dma_start(out=d["g1d"][t], in_=g1[:])
                nc.sync.dma_start(out=d["g2d"][t], in_=g2[:])

                # conv1 rhs must start at partition 0: DMA bands into a flat
                # [4, 8192] tile
                feat = fpool.tile([4, 8192], F32, tag="feat")
                for g in range(8):
                    nc.sync.dma_start(
                        out=_r(feat[0:3, g * 512:(g + 1) * 512]),
                        in_=_r(g1[16 * g: 16 * g + 3, :]))
                    nc.sync.dma_start(
                        out=_r(feat[0:3, (8 + g) * 512:(9 + g) * 512]),
                        in_=_r(g2[16 * g: 16 * g + 3, :]))

                # dist = sqrt(max(|q|^2 - val, 0)) into feat row 3
                d2 = wp.tile([128, 64], F32, tag="d2")
                nc.vector.tensor_scalar(
                    out=d2[:], in0=vals[:], scalar1=qsq[:, t:t + 1],
                    scalar2=-1.0, op0=OP.subtract, op1=OP.mult)
                nc.vector.tensor_scalar_max(d2[:], d2[:], 0.0)
                nc.scalar.activation(out=d2[:], in_=d2[:], func=AF.Sqrt)
                # shuffle dist to pixel layout: DVE 32x32 block-transpose,
                # then per-chunk DMAs that swap the block grid; value
                # d2[q, kn*32+s] sits at d2bt[32*(q//32)+s, 32*kn+(q%32)]
                d2bt = wp.tile([128, 64], F32, tag="d2bt")
                nc.vector.transpose(out=d2bt[:], in_=d2[:])
                for kn in (0, 1):
                    for g in range(8):
                        c = kn * 8 + g
                        nc.sync.dma_start(
                            out=_r(feat[3:4, c * 512:(c + 1) * 512]
                                   .rearrange("c (s p) -> c s p", s=32)),
                            in_=_r(d2bt[32 * (g // 2):32 * (g // 2) + 32,
                                        32 * kn + 16 * (g % 2):
                                        32 * kn + 16 * (g % 2) + 16]))

                # resi = nn - q (in place on coord rows)
                qrt = qr[0:3, t * 128:(t + 1) * 128]
                for kn in (0, 1):
                    nc.vector.tensor_tensor(
                        out=_r(feat[0:3, kn * 4096:(kn + 1) * 4096]
                               .rearrange("c (g s p) -> c g s p", g=8, s=32)),
                        in0=feat[0:3, kn * 4096:(kn + 1) * 4096]
                            .rearrange("c (g s p) -> c g s p", g=8, s=32),
                        in1=qrt.rearrange("c (g p) -> c g p", g=8)
                            .unsqueeze(2).to_broadcast([3, 8, 32, 16]),
                        op=OP.subtract)

                # conv1: 16 chunks in 4 psum groups of 4 -> y1 packed
                # [128, 4096]; group g: partition half h = g//2, free quarter
                # q = g%2, chunks 8h+4q+cc at psum free 512*cc
                y1 = yp.tile([128, 4096], F32, tag="y1")
                for g in range(4):
                    h, q = g // 2, g % 2
                    pc = cp.tile([C1, 2048], F32, tag="pc1")
                    for cc in range(4):
                        c = 8 * h + 4 * q + cc
                        nc.tensor.matmul(
                            out=pc[:, 512 * cc:512 * cc + 512],
                            lhsT=_r(w1[:]),
                            rhs=_r(feat[:, c * 512:(c + 1) * 512]),
                            start=True, stop=True)
                    nc.scalar.activation(
                        out=y1[64 * h:64 * h + 64,
                               2048 * q:2048 * (q + 1)], in_=pc[:],
                        func=AF.Identity,
                        accum_out=sm1[:, t * 4 + g: t * 4 + g + 1])
                    sqs = wp.tile([C1, 2048], F32, tag="sqs")
                    nc.scalar.activation(
                        out=sqs[:], in_=pc[:], func=AF.Square,
                        accum_out=sq1[:, t * 4 + g: t * 4 + g + 1])
                nc.sync.dma_start(out=d["y1d"][t], in_=y1[:])

        _bn_allreduce(tc, 0, sm1, sq1, gb1, ab1, d["arin0"], d["arout0"], True)

        # ---------------- Phase 2: apply BN1+relu, conv2 ----------------
        with tc.tile_pool(name="p2y", bufs=2) as yp, \
             tc.tile_pool(name="p2psum", bufs=2, space="PSUM") as cp, \
             tc.tile_pool(name="p2work", bufs=2) as wp:
            for t in range(NT):
                y1 = yp.tile([128, 4096], F32, tag="y1l")
                nc.sync.dma_start(out=y1[:], in_=d["y1d"][t])
                nc.scalar.activation(
                    out=_r(y1[:]), in_=y1[:], func=AF.Relu,
                    scale=ab1[:, 0:1], bias=ab1[:, 1:2])
                y2 = yp.tile([128, 4096], F32, tag="y2")
                for g in range(4):
                    h, q = g // 2, g % 2
                    pc = cp.tile([C2, 2048], F32, tag="pc2")
                    for cc in range(4):
                        fo = 2048 * q + 512 * cc
                        nc.tensor.matmul(
                            out=pc[:, 512 * cc:512 * cc + 512],
                            lhsT=_r(w2[64 * h:64 * h + 64, :]),
                            rhs=_r(y1[64 * h:64 * h + 64, fo:fo + 512]),
                            start=True, stop=True)
                    nc.scalar.activation(
                        out=y2[64 * h:64 * h + 64,
                               2048 * q:2048 * (q + 1)], in_=pc[:],
                        func=AF.Identity,
                        accum_out=sm2[:, t * 4 + g: t * 4 + g + 1])
                    sqs = wp.tile([C2, 2048], F32, tag="sqs2")
                    nc.scalar.activation(
                        out=sqs[:], in_=pc[:], func=AF.Square,
                        accum_out=sq2[:, t * 4 + g: t * 4 + g + 1])
                nc.sync.dma_start(out=d["y2d"][t], in_=y2[:])

        _bn_allreduce(tc, 1, sm2, sq2, gb2, ab2, d["arin1"], d["arout1"], True)

        # ---------------- Phase 3: apply BN2+relu, conv3 ----------------
        with tc.tile_pool(name="p3y", bufs=2) as yp, \
             tc.tile_pool(name="p3psum", bufs=2, space="PSUM") as cp, \
             tc.tile_pool(name="p3work", bufs=2) as wp:
            for t in range(NT):
                y2 = yp.tile([128, 4096], F32, tag="y2l")
                nc.sync.dma_start(out=y2[:], in_=d["y2d"][t])
                nc.scalar.activation(
                    out=_r(y2[:]), in_=y2[:], func=AF.Relu,
                    scale=ab2[:, 0:1], bias=ab2[:, 1:2])
                y3 = yp.tile([C3, 8192], F32, tag="y3")
                for g in range(4):
                    pc = cp.tile([128, 2048], F32, tag="pc3")
                    for cc in range(4):
                        c = 4 * g + cc
                        bp_, fo = _pk(c)
                        nc.tensor.matmul(
                            out=pc[:, 512 * cc:512 * cc + 512],
                            lhsT=_r(w3[bp_:bp_ + 64, :]),
                            rhs=_r(y2[bp_:bp_ + 64, fo:fo + 512]),
                            start=True, stop=True)
                    nc.scalar.activation(
                        out=y3[:, 2048 * g:2048 * (g + 1)], in_=pc[:],
                        func=AF.Identity,
                        accum_out=sm3[:, t * 4 + g: t * 4 + g + 1])
                    sqs = wp.tile([C3, 2048], F32, tag="sqs3")
                    nc.scalar.activation(
                        out=sqs[:], in_=pc[:], func=AF.Square,
                        accum_out=sq3[:, t * 4 + g: t * 4 + g + 1])
                nc.sync.dma_start(out=d["y3d"][t], in_=y3[:])

        _bn_allreduce(tc, 2, sm3, sq3, gb3, ab3, d["arin2"], d["arout2"], False)

        # ------------- Phase 4: scores, softmax, fusion, output -------------
        with tc.tile_pool(name="p4y", bufs=2) as yp, \
             tc.tile_pool(name="p4work", bufs=2) as wp, \
             tc.tile_pool(name="p4par", bufs=1) as parp, \
             tc.tile_pool(name="p4psum", bufs=2, space="PSUM") as pp4, \
             tc.tile_pool(name="p4out", bufs=1) as op_:
            outsb = op_.tile([4, QPC], F32)
            for t in range(NT):
                # channel-max scores, split by knn half (engine partition
                # bases must be 32-aligned, so rows land via DMA)
                scA = wp.tile([8, 512], F32, tag="scA")
                scB = wp.tile([8, 512], F32, tag="scB")
                for half, dst in ((0, scA), (1, scB)):
                    y3 = yp.tile([C3, 4096], F32, tag="y3l")
                    nc.sync.dma_start(
                        out=y3[:],
                        in_=d["y3d"][t][:, half * 4096:(half + 1) * 4096])
                    nc.scalar.activation(
                        out=y3[:], in_=y3[:], func=AF.Relu,
                        scale=ab3[:, 0:1], bias=ab3[:, 1:2])
                    par = parp.tile([128, 4096], F32, tag="par")
                    nc.gpsimd.partition_all_reduce(
                        out_ap=par[:], in_ap=y3[:],
                        channels=128, reduce_op=bass_isa.ReduceOp.max)
                    nc.sync.dma_start(
                        out=dst[:],
                        in_=par[0:1, :].rearrange("a (c f) -> a c f", c=8))
                # softmax over the 64 neighbors of each query
                qmA = wp.tile([8, 16], F32, tag="qmA")
                qmB = wp.tile([8, 16], F32, tag="qmB")
                for sct, qm in ((scA, qmA), (scB, qmB)):
                    nc.vector.tensor_reduce(
                        out=qm[:],
                        in_=sct[:].rearrange("c (s p) -> c p s", s=32),
                        axis=mybir.AxisListType.X, op=OP.max)
                nc.vector.tensor_tensor(
                    out=qmA[:], in0=qmA[:], in1=qmB[:], op=OP.max)
                exA = wp.tile([8, 512], F32, tag="exA")
                exB = wp.tile([8, 512], F32, tag="exB")
                exrA = wp.tile([8, 512], F32, tag="exrA")
                exrB = wp.tile([8, 512], F32, tag="exrB")
                for sct, ext, exr in ((scA, exA, exrA), (scB, exB, exrB)):
                    nc.vector.tensor_tensor(
                        out=ext[:].rearrange("c (s p) -> c s p", s=32),
                        in0=sct[:].rearrange("c (s p) -> c s p", s=32),
                        in1=qmA[:].unsqueeze(1).to_broadcast([8, 32, 16]),
                        op=OP.subtract)
                    nc.scalar.activation(out=_r(exr[:]), in_=ext[:],
                                         func=AF.Exp)
                esA = wp.tile([8, 16], F32, tag="esA")
                esB = wp.tile([8, 16], F32, tag="esB")
                for ext, est in ((exrA, esA), (exrB, esB)):
                    nc.vector.tensor_reduce(
                        out=est[:],
                        in_=ext[:].rearrange("c (s p) -> c p s", s=32),
                        axis=mybir.AxisListType.X, op=OP.add)
                nc.vector.tensor_tensor(
                    out=esA[:], in0=esA[:], in1=esB[:], op=OP.add)
                nc.vector.reciprocal(out=esA[:], in_=esA[:])
                esr = wp.tile([8, 16], F32, tag="esr")
                nc.scalar.activation(out=_r(esr[:]), in_=esA[:],
                                     func=AF.Identity)
                # fusion: replicate unnormalized weight rows onto band
                # partitions via a selector matmul, multiply with raw coords,
                # segment-reduce, then scale by the replicated 1/Z
                g1 = wp.tile([128, 512], F32, tag="g1l")
                g2 = wp.tile([128, 512], F32, tag="g2l")
                nc.sync.dma_start(out=g1[:], in_=d["g1d"][t])
                nc.sync.dma_start(out=g2[:], in_=d["g2d"][t])
                wr1 = wp.tile([128, 512], F32, tag="wr1")
                wr2 = wp.tile([128, 512], F32, tag="wr2")
                for ext, wr in ((exrA, wr1), (exrB, wr2)):
                    pw = pp4.tile([128, 512], F32, tag="pw")
                    nc.tensor.matmul(
                        out=pw[:], lhsT=_r(selw[:]),
                        rhs=_r(ext[:]), start=True, stop=True)
                    nc.scalar.activation(out=wr[:], in_=pw[:], func=AF.Identity)
                pwz = pp4.tile([128, 16], F32, tag="pwz")
                nc.tensor.matmul(
                    out=pwz[:], lhsT=_r(selw[:]), rhs=_r(esr[:]),
                    start=True, stop=True)
                zr = wp.tile([128, 16], F32, tag="zr")
                nc.scalar.activation(out=zr[:], in_=pwz[:], func=AF.Identity)
                pr = wp.tile([128, 512], F32, tag="pr")
                nc.vector.tensor_tensor(out=pr[:], in0=g1[:], in1=wr1[:],
                                        op=OP.mult)
                nc.vector.tensor_tensor(out=wr2[:], in0=g2[:], in1=wr2[:],
                                        op=OP.mult)
                nc.vector.tensor_tensor(out=pr[:], in0=pr[:], in1=wr2[:],
                                        op=OP.add)
                fp = wp.tile([128, 16], F32, tag="fp")
                nc.vector.tensor_reduce(
                    out=fp[:], in_=pr[:].rearrange("c (s p) -> c p s", s=32),
                    axis=mybir.AxisListType.X, op=OP.add)
                nc.vector.tensor_tensor(out=fp[:], in0=fp[:], in1=zr[:],
                                        op=OP.mult)
                for g in range(8):
                    nc.sync.dma_start(
                        out=outsb[0:3,
                                  t * 128 + 16 * g: t * 128 + 16 * g + 16],
                        in_=fp[16 * g: 16 * g + 3, :])
            nc.sync.dma_start(out=d["out"][:], in_=outsb[0:3, :])


def _bn_allreduce(tc, li, sm, sq, gbe, ab, arin, arout, dup):
    """Reduce per-chunk stat slots, AllReduce across 8 cores, compute
    per-channel scale a = g*rsqrt(var+eps) and bias b = be - a*mean.
    When the stat rows are packed [128 = 2x64ch] (dup layers), fold the
    upper half into the lower before the reduce."""
    nc = tc.nc
    C = gbe.shape[0]
    with tc.tile_pool(name=f"bn{li}", bufs=1) as bp:
        st = bp.tile([C, 2], F32)
        nc.vector.tensor_reduce(out=st[:, 0:1], in_=sm[:],
                                axis=mybir.AxisListType.X, op=OP.add)
        nc.vector.tensor_reduce(out=st[:, 1:2], in_=sq[:],
                                axis=mybir.AxisListType.X, op=OP.add)
        nc.sync.dma_start(out=arin[:], in_=st[:])
        if getattr(nc, "_single_core_nocoll", False):
            nc.sync.dma_start(out=arout[:], in_=arin[:])
        else:
            nc.gpsimd.collective_compute(
                "AllReduce", OP.add, replica_groups=[list(range(NCORES))],
                ins=[arin.opt()], outs=[arout.opt()])
        ar = bp.tile([C, 2], F32)
        nc.sync.dma_start(out=ar[:], in_=arout[:])
        mean = bp.tile([C, 1], F32)
        var = bp.tile([C, 1], F32)
        nc.vector.tensor_scalar_mul(mean[:], ar[:, 0:1], 1.0 / NTOT)
        nc.vector.tensor_scalar_mul(var[:], ar[:, 1:2], 1.0 / NTOT)
        m2 = bp.tile([C, 1], F32)
        nc.vector.tensor_tensor(out=m2[:], in0=mean[:], in1=mean[:], op=OP.mult)
        nc.vector.tensor_tensor(out=var[:], in0=var[:], in1=m2[:], op=OP.subtract)
        nc.vector.tensor_scalar_add(var[:], var[:], BN_EPS)
        nc.scalar.activation(out=var[:], in_=var[:], func=AF.Sqrt)
        nc.vector.reciprocal(out=var[:], in_=var[:])  # rsqrt(var+eps)
        nc.vector.tensor_tensor(out=ab[0:C, 0:1], in0=var[:], in1=gbe[:, 0:1],
                                op=OP.mult)            # a
        nc.vector.tensor_tensor(out=m2[:], in0=ab[0:C, 0:1], in1=mean[:],
                                op=OP.mult)
        nc.vector.tensor_tensor(out=ab[0:C, 1:2], in0=gbe[:, 1:2], in1=m2[:],
                                op=OP.subtract)        # b = be - a*mean
        if dup:
            nc.vector.tensor_copy(out=ab[C:2 * C, :], in_=ab[0:C, :])


_PROGRAM = None
LAST_RESULT = None


def _get_program():
    global _PROGRAM
    if _PROGRAM is None:
        _PROGRAM = _build_program()
    return _PROGRAM


def _prep_core_inputs(points1, points2, W1, W2, W3, gs, bes, b, h):
    p1 = points1[b]          # [3, N]
    p2 = points2[b]
    q = p1[:, h * QPC:(h + 1) * QPC]            # [3, QPC]
    qf = np.concatenate([2.0 * q, np.ones((1, QPC), np.float32)], axis=0)

    def cand_tab(p):
        sq = (p * p).sum(axis=0, keepdims=True)
        return np.concatenate([p, -sq], axis=0).astype(np.float32)  # [4, N]

    gtab = np.zeros((128, N), np.float32)
    gtab2 = np.zeros((128, N), np.float32)
    for g in range(8):
        gtab[16 * g + 0:16 * g + 3] = p1
        gtab2[16 * g + 0:16 * g + 3] = p2
    qraw = np.zeros((4, QPC), np.float32)
    qraw[0:3] = q
    qsqv = (q * q).sum(axis=0).reshape(NT, 128).T.astype(np.float32)  # [128, NT]

    def dup128(w):      # [64, C] -> [128, C] duplicated
        return np.concatenate([w, w], axis=0).astype(np.float32)

    selw = np.zeros((8, 128), np.float32)
    for g in range(8):
        for c3 in range(3):
            selw[g, 16 * g + c3] = 1.0

    return {
        "selw": selw,
        "qf": qf.astype(np.float32),
        "t1": cand_tab(p1), "t2": cand_tab(p2), "gt": gtab, "gt2": gtab2,
        "qr": qraw, "qsq": np.ascontiguousarray(qsqv),
        "w1t": np.ascontiguousarray(W1.T).astype(np.float32),
        "w2t": dup128(np.ascontiguousarray(W2.T)),
        "w3t": dup128(np.ascontiguousarray(W3.T)),
        "gb1": np.stack([gs[0], bes[0]], axis=1).astype(np.float32),
        "gb2": np.stack([gs[1], bes[1]], axis=1).astype(np.float32),
        "gb3": np.stack([gs[2], bes[2]], axis=1).astype(np.float32),
    }


def kernel(points1, points2, k, t, W1, b1, g1, be1, W2, b2, g2, be2,
           W3, b3, g3, be3):
    # b1/b2/b3 cancel inside train-mode BatchNorm; t is unused by the net.
    assert int(np.asarray(k)) == KNN
    points1 = np.asarray(points1, np.float32)
    points2 = np.asarray(points2, np.float32)
    gs = [np.asarray(g1, np.float32), np.asarray(g2, np.float32),
          np.asarray(g3, np.float32)]
    bes = [np.asarray(be1, np.float32), np.asarray(be2, np.float32),
           np.asarray(be3, np.float32)]
    Ws = [np.asarray(W1, np.float32), np.asarray(W2, np.float32),
          np.asarray(W3, np.float32)]

    in_maps = []
    for c in range(NCORES):
        b, h = divmod(c, 2)
        in_maps.append(_prep_core_inputs(points1, points2, *Ws, gs, bes, b, h))

    nc = _get_program()
    bkr = run_bass_kernel_spmd(nc, in_maps, list(range(NCORES)))
    global LAST_RESULT
    LAST_RESULT = bkr
    res = bkr.results

    out = np.zeros((B, 3, N), np.float32)
    for c in range(NCORES):
        b, h = divmod(c, 2)
        out[b, :, h * QPC:(h + 1) * QPC] = res[c]["out"]
    return out



# revision 62
# speedup vs baseline: 1.2710x; 1.0943x over previous
"""PointsFusion Trainium2 kernel.

Pipeline per batch b (B=4, N=4096, k=32):
  knn1 = 32-NN of p1 in p1, knn2 = 32-NN of p1 in p2 (exact, via DVE 8-max rounds)
  gather neighbor coords, features (resi, dist) -> conv(4->64)->BN->relu
  -> conv(64->64)->BN->relu -> conv(64->128)->BN->relu -> channel-max scores
  -> softmax over 64 neighbors -> weighted sum of neighbor coords.

Sharding: 8 cores = (batch b, half h of the 4096 query points). BatchNorm uses
global batch stats -> 3 tiny AllReduces of per-channel sum/sumsq.

Layouts (per 128-query tile):
  pixel space: 16 chunks of 512; chunk c = kn*8+g, pixel j = c*512 + s*16 + p
  (g = query group, p = query-in-group, s = neighbor slot, kn = which knn).
  64-channel activations are packed [128, 4096]: chunk c lives at partitions
  64*(c%2)..+64, free 512*(c//2)..+512 (keeps matmul rhs bases in {0, 64}).

Self-contained: hardcodes shapes; no sibling imports.
"""

import sys

import numpy as np

for _p in ("/opt/trn_rl_repo", "/opt/pypackages"):
    if _p not in sys.path:
        sys.path.append(_p)

import concourse.bass as bass  # noqa: E402  (imported for side effects/typing)  # noqa: F401
import concourse.mybir as mybir  # noqa: E402
import concourse.tile as tile  # noqa: E402
from concourse import bacc, bass_isa  # noqa: E402
from concourse.bass_utils import run_bass_kernel_spmd  # noqa: E402

F32 = mybir.dt.float32
F32R = mybir.dt.float32r
U16 = mybir.dt.uint16
I16 = mybir.dt.int16
I32 = mybir.dt.int32
AF = mybir.ActivationFunctionType
OP = mybir.AluOpType


USE_F32R = True


def _r(ap):
    """view an f32 AP as f32r for full-rate PE streaming"""
    return ap.bitcast(F32R) if USE_F32R else ap

NCORES = 8
B = 4
N = 4096          # candidate points per batch
KNN = 32          # neighbors per knn
QPC = 2048        # query points per core
NT = 16           # query tiles of 128 per core
C1, C2, C3 = 64, 64, 128
NTOT = float(B * N * 2 * KNN)   # BN stat count (global)
BN_EPS = 1e-3
NEG = -1.0e30


def _pk(cc):
    """packed [128, 4096] slice coords for chunk cc (chunks 0-7 on the lower
    partition half, 8-15 on the upper; matmul PSUM outs stay at base 0)."""
    return 64 * (cc // 8), 512 * (cc % 8)


def _build_program(single=False):
    nc = bacc.Bacc(
        "TRN2", target_bir_lowering=False, debug=False,
        num_devices=1 if single else NCORES,
    )
    nc._single_core_nocoll = single

    ap = {}
    def din(name, shape):
        ap[name] = nc.dram_tensor(name, shape, F32, kind="ExternalInput").ap()
    din("qf", [4, QPC])
    din("gt", [128, N])
    din("qr", [4, QPC])
    din("qsq", [128, NT])
    din("w1t", [4, C1])
    din("w2t", [128, C2])     # duplicated at partition 64
    din("w3t", [128, C3])     # duplicated at partition 64
    din("gt2", [128, N])
    din("gb1", [C1, 2])
    din("gb2", [C2, 2])
    din("gb3", [C3, 2])
    din("selw", [8, 128])

    ap["out"] = nc.dram_tensor("out", [3, QPC], F32, kind="ExternalOutput").ap()

    ap["y1d"] = nc.dram_tensor("y1d", [NT, 128, 4096], F32).ap()
    ap["y2d"] = nc.dram_tensor("y2d", [NT, 128, 4096], F32).ap()
    ap["y3d"] = nc.dram_tensor("y3d", [NT, C3, 8192], F32).ap()
    ap["g1d"] = nc.dram_tensor("g1d", [NT, 128, 512], F32).ap()
    ap["g2d"] = nc.dram_tensor("g2d", [NT, 128, 512], F32).ap()
    for i, c in ((0, C1), (1, C2), (2, C3)):
        ap[f"arin{i}"] = nc.dram_tensor(f"arin{i}", [c * 2], F32).ap()
        ap[f"arout{i}"] = nc.dram_tensor(f"arout{i}", [c * 2], F32).ap()

    with tile.TileContext(nc) as tc:
        _kernel_body(tc, ap)
    nc.compile()
    return nc


def _kernel_body(tc, d):
    nc = tc.nc
    from contextlib import ExitStack

    ctx = ExitStack()
    with ctx:
        cpool = ctx.enter_context(tc.tile_pool(name="consts", bufs=1))
        gt = cpool.tile([128, N], F32)
        qf = cpool.tile([4, QPC], F32)
        qr = cpool.tile([4, QPC], F32)
        qsq = cpool.tile([128, NT], F32)
        w1 = cpool.tile([4, C1], F32)
        w2 = cpool.tile([128, C2], F32)
        w3 = cpool.tile([128, C3], F32)
        gb1 = cpool.tile([C1, 2], F32)
        gb2 = cpool.tile([C2, 2], F32)
        gb3 = cpool.tile([C3, 2], F32)
        gt2 = cpool.tile([128, N], F32)
        selw = cpool.tile([8, 128], F32)
        # tiles consumed by fp32r matmuls get f32r-typed producer DMAs so the
        # BIR verifier sees a consistently-rounded chain
        r_consts = {"w1t", "w2t", "w3t", "selw"}
        for nm, sb in [("gt", gt), ("gt2", gt2),
                       ("qf", qf),
                       ("qr", qr), ("qsq", qsq), ("w1t", w1), ("w2t", w2),
                       ("w3t", w3), ("gb1", gb1), ("gb2", gb2), ("gb3", gb3),
                       ("selw", selw)]:
            if nm in r_consts:
                nc.sync.dma_start(out=_r(sb[:]), in_=_r(d[nm][:]))
            else:
                nc.sync.dma_start(out=sb[:], in_=d[nm][:])

        spool = ctx.enter_context(tc.tile_pool(name="stats", bufs=1))
        sm1 = spool.tile([C1, NT * 4], F32)
        sq1 = spool.tile([C1, NT * 4], F32)
        sm2 = spool.tile([C2, NT * 4], F32)
        sq2 = spool.tile([C2, NT * 4], F32)
        sm3 = spool.tile([C3, NT * 4], F32)
        sq3 = spool.tile([C3, NT * 4], F32)
        ab1 = spool.tile([128, 2], F32)   # col0 = scale a, col1 = bias b (dup at 64)
        ab2 = spool.tile([128, 2], F32)
        ab3 = spool.tile([C3, 2], F32)

        # ---------------- Phase 1: knn + gather + feat + conv1 ----------------
        with tc.tile_pool(name="p1m", bufs=2) as mpool, \
             tc.tile_pool(name="p1psum", bufs=1, space="PSUM") as pp, \
             tc.tile_pool(name="p1cpsum", bufs=1, space="PSUM") as cp, \
             tc.tile_pool(name="p1feat", bufs=1) as fpool, \
             tc.tile_pool(name="p1work", bufs=2) as wp, \
             tc.tile_pool(name="p1y", bufs=2) as yp:
            for t in range(NT):
                # Exact top-32 per table, block-hierarchically: top-16 of
                # each 512-block via MAX8/MATCH_REPLACE8 on a scratch copy,
                # 128-wide merge, then FIND_INDEX8 of the 32 winners against
                # the pristine score row (4 finds of 4096 instead of scans
                # of 4096 for every MAX8/REPLACE round).
                msb = mpool.tile([128, N], F32, tag="msb")
                ms2 = mpool.tile([128, N], F32, tag="ms2")
                vals = wp.tile([128, 64], F32, tag="vals")
                idxu = wp.tile([128, 64], U16, tag="idxu")
                idxi = wp.tile([128, 64], I16, tag="idxi")
                for kn, tab in ((0, gt), (1, gt2)):
                    # M = 2 q.c - |c|^2 (maximize == nearest); fp32 matmul
                    for ch in range(2):
                        pm = pp.tile([128, 2048], F32, tag="pm")
                        for h in range(4):
                            nc.tensor.matmul(
                                out=pm[:, h * 512:(h + 1) * 512],
                                lhsT=qf[:, t * 128:(t + 1) * 128],
                                rhs=tab[0:4, (4 * ch + h) * 512:
                                        (4 * ch + h + 1) * 512],
                                start=True, stop=True,
                            )
                        nc.scalar.activation(
                            out=msb[:, ch * 2048:(ch + 1) * 2048], in_=pm[:],
                            func=AF.Identity)
                        nc.scalar.activation(
                            out=ms2[:, ch * 2048:(ch + 1) * 2048], in_=pm[:],
                            func=AF.Identity)
                    mrg = wp.tile([128, 128], F32, tag="mrg")
                    for b in range(8):
                        mb = ms2[:, 512 * b:512 * (b + 1)]
                        nc.vector.max(out=mrg[:, 16 * b:16 * b + 8], in_=mb)
                        nc.vector.match_replace(
                            out=mb, in_to_replace=mrg[:, 16 * b:16 * b + 8],
                            in_values=mb, imm_value=NEG)
                        nc.vector.max(out=mrg[:, 16 * b + 8:16 * b + 16],
                                      in_=mb)
                    for r in range(4):
                        v8 = vals[:, kn * 32 + r * 8: kn * 32 + r * 8 + 8]
                        i8 = idxu[:, kn * 32 + r * 8: kn * 32 + r * 8 + 8]
                        nc.vector.max(out=v8, in_=mrg[:])
                        nc.vector.max_index(out=i8, in_max=v8,
                                            in_values=msb[:])
                        if r < 3:
                            nc.vector.match_replace(
                                out=mrg[:], in_to_replace=v8,
                                in_values=mrg[:], imm_value=NEG)
                    # duplicate f32 scores straddling a find-round boundary
                    # both resolve to the first occurrence (FIND_INDEX8 only
                    # advances over dups within one call); refind the three
                    # boundary pairs in a single call and patch the second
                    # of each pair
                    bnd = wp.tile([128, 8], F32, tag="bnd")
                    f8b = wp.tile([128, 8], U16, tag="f8b")
                    for i, pos in enumerate((7, 15, 23)):
                        nc.vector.tensor_copy(
                            out=bnd[:, 2 * i:2 * i + 2],
                            in_=vals[:, kn * 32 + pos:kn * 32 + pos + 2])
                    nc.vector.tensor_copy(out=bnd[:, 6:8],
                                          in_=vals[:, kn * 32 + 30:
                                                    kn * 32 + 32])
                    nc.vector.max_index(out=f8b[:], in_max=bnd[:],
                                        in_values=msb[:])
                    for i, pos in enumerate((8, 16, 24)):
                        nc.vector.tensor_copy(
                            out=idxu[:, kn * 32 + pos:kn * 32 + pos + 1],
                            in_=f8b[:, 2 * i + 1:2 * i + 2])
                nc.vector.tensor_copy(out=idxi[:], in_=idxu[:])
                # gather neighbor coords; both tables carry xyz on band rows
                # 16g+{0..2} (gt = p1 for knn1, gt2 = p2 for knn2); spill raw
                # for the fusion phase
                g1 = wp.tile([128, 512], F32, tag="g1")
                g2 = wp.tile([128, 512], F32, tag="g2")
                nc.gpsimd.ap_gather(
                    out_ap=g1[:], in_ap=gt[:], idxs_ap=idxi[:, 0:32],
                    channels=128, num_elems=N, d=1, num_idxs=512)
                nc.gpsimd.ap_gather(
                    out_ap=g2[:], in_ap=gt2[:], idxs_ap=idxi[:, 32:64],
                    channels=128, num_elems=N, d=1, num_idxs=512)
                nc.sync.dma_start(out=d["g1d"][t], in_=g1[:])
                nc.sync.dma_start(out=d["g2d"][t], in_=g2[:])

                # conv1 rhs must start at partition 0: DMA bands into a flat
                # [4, 8192] tile
                feat = fpool.tile([4, 8192], F32, tag="feat")
                for g in range(8):
                    nc.sync.dma_start(
                        out=_r(feat[0:3, g * 512:(g + 1) * 512]),
                        in_=_r(g1[16 * g: 16 * g + 3, :]))
                    nc.sync.dma_start(
                        out=_r(feat[0:3, (8 + g) * 512:(9 + g) * 512]),
                        in_=_r(g2[16 * g: 16 * g + 3, :]))

                # dist = sqrt(max(|q|^2 - val, 0)) into feat row 3
                d2 = wp.tile([128, 64], F32, tag="d2")
                nc.vector.tensor_scalar(
                    out=d2[:], in0=vals[:], scalar1=qsq[:, t:t + 1],
                    scalar2=-1.0, op0=OP.subtract, op1=OP.mult)
                nc.vector.tensor_scalar_max(d2[:], d2[:], 0.0)
                nc.scalar.activation(out=d2[:], in_=d2[:], func=AF.Sqrt)
                # shuffle dist to pixel layout: DVE 32x32 block-transpose,
                # then per-chunk DMAs that swap the block grid; value
                # d2[q, kn*32+s] sits at d2bt[32*(q//32)+s, 32*kn+(q%32)]
                d2bt = wp.tile([128, 64], F32, tag="d2bt")
                nc.vector.transpose(out=d2bt[:], in_=d2[:])
                for kn in (0, 1):
                    for g in range(8):
                        c = kn * 8 + g
                        nc.sync.dma_start(
                            out=_r(feat[3:4, c * 512:(c + 1) * 512]
                                   .rearrange("c (s p) -> c s p", s=32)),
                            in_=_r(d2bt[32 * (g // 2):32 * (g // 2) + 32,
                                        32 * kn + 16 * (g % 2):
                                        32 * kn + 16 * (g % 2) + 16]))

                # resi = nn - q (in place on coord rows)
                qrt = qr[0:3, t * 128:(t + 1) * 128]
                for kn in (0, 1):
                    nc.vector.tensor_tensor(
                        out=_r(feat[0:3, kn * 4096:(kn + 1) * 4096]
                               .rearrange("c (g s p) -> c g s p", g=8, s=32)),
                        in0=feat[0:3, kn * 4096:(kn + 1) * 4096]
                            .rearrange("c (g s p) -> c g s p", g=8, s=32),
                        in1=qrt.rearrange("c (g p) -> c g p", g=8)
                            .unsqueeze(2).to_broadcast([3, 8, 32, 16]),
                        op=OP.subtract)

                # conv1: 16 chunks in 4 psum groups of 4 -> y1 packed
                # [128, 4096]; group g: partition half h = g//2, free quarter
                # q = g%2, chunks 8h+4q+cc at psum free 512*cc
                y1 = yp.tile([128, 4096], F32, tag="y1")
                for g in range(4):
                    h, q = g // 2, g % 2
                    pc = cp.tile([C1, 2048], F32, tag="pc1")
                    for cc in range(4):
                        c = 8 * h + 4 * q + cc
                        nc.tensor.matmul(
                            out=pc[:, 512 * cc:512 * cc + 512],
                            lhsT=_r(w1[:]),
                            rhs=_r(feat[:, c * 512:(c + 1) * 512]),
                            start=True, stop=True)
                    nc.scalar.activation(
                        out=y1[64 * h:64 * h + 64,
                               2048 * q:2048 * (q + 1)], in_=pc[:],
                        func=AF.Identity,
                        accum_out=sm1[:, t * 4 + g: t * 4 + g + 1])
                    sqs = wp.tile([C1, 2048], F32, tag="sqs")
                    nc.scalar.activation(
                        out=sqs[:], in_=pc[:], func=AF.Square,
                        accum_out=sq1[:, t * 4 + g: t * 4 + g + 1])
                nc.sync.dma_start(out=d["y1d"][t], in_=y1[:])

        _bn_allreduce(tc, 0, sm1, sq1, gb1, ab1, d["arin0"], d["arout0"], True)

        # ---------------- Phase 2: apply BN1+relu, conv2 ----------------
        with tc.tile_pool(name="p2y", bufs=2) as yp, \
             tc.tile_pool(name="p2psum", bufs=2, space="PSUM") as cp, \
             tc.tile_pool(name="p2work", bufs=2) as wp:
            for t in range(NT):
                y1 = yp.tile([128, 4096], F32, tag="y1l")
                nc.sync.dma_start(out=y1[:], in_=d["y1d"][t])
                nc.scalar.activation(
                    out=_r(y1[:]), in_=y1[:], func=AF.Relu,
                    scale=ab1[:, 0:1], bias=ab1[:, 1:2])
                y2 = yp.tile([128, 4096], F32, tag="y2")
                for g in range(4):
                    h, q = g // 2, g % 2
                    pc = cp.tile([C2, 2048], F32, tag="pc2")
                    for cc in range(4):
                        fo = 2048 * q + 512 * cc
                        nc.tensor.matmul(
                            out=pc[:, 512 * cc:512 * cc + 512],
                            lhsT=_r(w2[64 * h:64 * h + 64, :]),
                            rhs=_r(y1[64 * h:64 * h + 64, fo:fo + 512]),
                            start=True, stop=True)
                    nc.scalar.activation(
                        out=y2[64 * h:64 * h + 64,
                               2048 * q:2048 * (q + 1)], in_=pc[:],
                        func=AF.Identity,
                        accum_out=sm2[:, t * 4 + g: t * 4 + g + 1])
                    sqs = wp.tile([C2, 2048], F32, tag="sqs2")
                    nc.scalar.activation(
                        out=sqs[:], in_=pc[:], func=AF.Square,
                        accum_out=sq2[:, t * 4 + g: t * 4 + g + 1])
                nc.sync.dma_start(out=d["y2d"][t], in_=y2[:])

        _bn_allreduce(tc, 1, sm2, sq2, gb2, ab2, d["arin1"], d["arout1"], True)

        # ---------------- Phase 3: apply BN2+relu, conv3 ----------------
        with tc.tile_pool(name="p3y", bufs=2) as yp, \
             tc.tile_pool(name="p3psum", bufs=2, space="PSUM") as cp, \
             tc.tile_pool(name="p3work", bufs=2) as wp:
            for t in range(NT):
                y2 = yp.tile([128, 4096], F32, tag="y2l")
                nc.sync.dma_start(out=y2[:], in_=d["y2d"][t])
                nc.scalar.activation(
                    out=_r(y2[:]), in_=y2[:], func=AF.Relu,
                    scale=ab2[:, 0:1], bias=ab2[:, 1:2])
                y3 = yp.tile([C3, 8192], F32, tag="y3")
                for g in range(4):
                    pc = cp.tile([128, 2048], F32, tag="pc3")
                    for cc in range(4):
                        c = 4 * g + cc
                        bp_, fo = _pk(c)
                        nc.tensor.matmul(
                            out=pc[:, 512 * cc:512 * cc + 512],
                            lhsT=_r(w3[bp_:bp_ + 64, :]),
                            rhs=_r(y2[bp_:bp_ + 64, fo:fo + 512]),
                            start=True, stop=True)
                    nc.scalar.activation(
                        out=y3[:, 2048 * g:2048 * (g + 1)], in_=pc[:],
                        func=AF.Identity,
                        accum_out=sm3[:, t * 4 + g: t * 4 + g + 1])
                    sqs = wp.tile([C3, 2048], F32, tag="sqs3")
                    nc.scalar.activation(
                        out=sqs[:], in_=pc[:], func=AF.Square,
                        accum_out=sq3[:, t * 4 + g: t * 4 + g + 1])
                nc.sync.dma_start(out=d["y3d"][t], in_=y3[:])

        _bn_allreduce(tc, 2, sm3, sq3, gb3, ab3, d["arin2"], d["arout2"], False)

        # ------------- Phase 4: scores, softmax, fusion, output -------------
        with tc.tile_pool(name="p4y", bufs=2) as yp, \
             tc.tile_pool(name="p4work", bufs=2) as wp, \
             tc.tile_pool(name="p4par", bufs=1) as parp, \
             tc.tile_pool(name="p4psum", bufs=2, space="PSUM") as pp4, \
             tc.tile_pool(name="p4out", bufs=1) as op_:
            outsb = op_.tile([4, QPC], F32)
            for t in range(NT):
                # channel-max scores, split by knn half (engine partition
                # bases must be 32-aligned, so rows land via DMA)
                scA = wp.tile([8, 512], F32, tag="scA")
                scB = wp.tile([8, 512], F32, tag="scB")
                for half, dst in ((0, scA), (1, scB)):
                    y3 = yp.tile([C3, 4096], F32, tag="y3l")
                    nc.sync.dma_start(
                        out=y3[:],
                        in_=d["y3d"][t][:, half * 4096:(half + 1) * 4096])
                    nc.scalar.activation(
                        out=y3[:], in_=y3[:], func=AF.Relu,
                        scale=ab3[:, 0:1], bias=ab3[:, 1:2])
                    par = parp.tile([128, 4096], F32, tag="par")
                    nc.gpsimd.partition_all_reduce(
                        out_ap=par[:], in_ap=y3[:],
                        channels=128, reduce_op=bass_isa.ReduceOp.max)
                    nc.sync.dma_start(
                        out=dst[:],
                        in_=par[0:1, :].rearrange("a (c f) -> a c f", c=8))
                # softmax over the 64 neighbors of each query
                qmA = wp.tile([8, 16], F32, tag="qmA")
                qmB = wp.tile([8, 16], F32, tag="qmB")
                for sct, qm in ((scA, qmA), (scB, qmB)):
                    nc.vector.tensor_reduce(
                        out=qm[:],
                        in_=sct[:].rearrange("c (s p) -> c p s", s=32),
                        axis=mybir.AxisListType.X, op=OP.max)
                nc.vector.tensor_tensor(
                    out=qmA[:], in0=qmA[:], in1=qmB[:], op=OP.max)
                exA = wp.tile([8, 512], F32, tag="exA")
                exB = wp.tile([8, 512], F32, tag="exB")
                exrA = wp.tile([8, 512], F32, tag="exrA")
                exrB = wp.tile([8, 512], F32, tag="exrB")
                for sct, ext, exr in ((scA, exA, exrA), (scB, exB, exrB)):
                    nc.vector.tensor_tensor(
                        out=ext[:].rearrange("c (s p) -> c s p", s=32),
                        in0=sct[:].rearrange("c (s p) -> c s p", s=32),
                        in1=qmA[:].unsqueeze(1).to_broadcast([8, 32, 16]),
                        op=OP.subtract)
                    nc.scalar.activation(out=_r(exr[:]), in_=ext[:],
                                         func=AF.Exp)
                esA = wp.tile([8, 16], F32, tag="esA")
                esB = wp.tile([8, 16], F32, tag="esB")
                for ext, est in ((exrA, esA), (exrB, esB)):
                    nc.vector.tensor_reduce(
                        out=est[:],
                        in_=ext[:].rearrange("c (s p) -> c p s", s=32),
                        axis=mybir.AxisListType.X, op=OP.add)
                nc.vector.tensor_tensor(
                    out=esA[:], in0=esA[:], in1=esB[:], op=OP.add)
                nc.vector.reciprocal(out=esA[:], in_=esA[:])
                esr = wp.tile([8, 16], F32, tag="esr")
                nc.scalar.activation(out=_r(esr[:]), in_=esA[:],
                                     func=AF.Identity)
                # fusion: replicate unnormalized weight rows onto band
                # partitions via a selector matmul, multiply with raw coords,
                # segment-reduce, then scale by the replicated 1/Z
                g1 = wp.tile([128, 512], F32, tag="g1l")
                g2 = wp.tile([128, 512], F32, tag="g2l")
                nc.sync.dma_start(out=g1[:], in_=d["g1d"][t])
                nc.sync.dma_start(out=g2[:], in_=d["g2d"][t])
                wr1 = wp.tile([128, 512], F32, tag="wr1")
                wr2 = wp.tile([128, 512], F32, tag="wr2")
                for ext, wr in ((exrA, wr1), (exrB, wr2)):
                    pw = pp4.tile([128, 512], F32, tag="pw")
                    nc.tensor.matmul(
                        out=pw[:], lhsT=_r(selw[:]),
                        rhs=_r(ext[:]), start=True, stop=True)
                    nc.scalar.activation(out=wr[:], in_=pw[:], func=AF.Identity)
                pwz = pp4.tile([128, 16], F32, tag="pwz")
                nc.tensor.matmul(
                    out=pwz[:], lhsT=_r(selw[:]), rhs=_r(esr[:]),
                    start=True, stop=True)
                zr = wp.tile([128, 16], F32, tag="zr")
                nc.scalar.activation(out=zr[:], in_=pwz[:], func=AF.Identity)
                pr = wp.tile([128, 512], F32, tag="pr")
                nc.vector.tensor_tensor(out=pr[:], in0=g1[:], in1=wr1[:],
                                        op=OP.mult)
                nc.vector.tensor_tensor(out=wr2[:], in0=g2[:], in1=wr2[:],
                                        op=OP.mult)
                nc.vector.tensor_tensor(out=pr[:], in0=pr[:], in1=wr2[:],
                                        op=OP.add)
                fp = wp.tile([128, 16], F32, tag="fp")
                nc.vector.tensor_reduce(
                    out=fp[:], in_=pr[:].rearrange("c (s p) -> c p s", s=32),
                    axis=mybir.AxisListType.X, op=OP.add)
                nc.vector.tensor_tensor(out=fp[:], in0=fp[:], in1=zr[:],
                                        op=OP.mult)
                for g in range(8):
                    nc.sync.dma_start(
                        out=outsb[0:3,
                                  t * 128 + 16 * g: t * 128 + 16 * g + 16],
                        in_=fp[16 * g: 16 * g + 3, :])
            nc.sync.dma_start(out=d["out"][:], in_=outsb[0:3, :])


def _bn_allreduce(tc, li, sm, sq, gbe, ab, arin, arout, dup):
    """Reduce per-chunk stat slots, AllReduce across 8 cores, compute
    per-channel scale a = g*rsqrt(var+eps) and bias b = be - a*mean.
    When the stat rows are packed [128 = 2x64ch] (dup layers), fold the
    upper half into the lower before the reduce."""
    nc = tc.nc
    C = gbe.shape[0]
    with tc.tile_pool(name=f"bn{li}", bufs=1) as bp:
        st = bp.tile([C, 2], F32)
        nc.vector.tensor_reduce(out=st[:, 0:1], in_=sm[:],
                                axis=mybir.AxisListType.X, op=OP.add)
        nc.vector.tensor_reduce(out=st[:, 1:2], in_=sq[:],
                                axis=mybir.AxisListType.X, op=OP.add)
        nc.sync.dma_start(out=arin[:], in_=st[:])
        if getattr(nc, "_single_core_nocoll", False):
            nc.sync.dma_start(out=arout[:], in_=arin[:])
        else:
            nc.gpsimd.collective_compute(
                "AllReduce", OP.add, replica_groups=[list(range(NCORES))],
                ins=[arin.opt()], outs=[arout.opt()])
        ar = bp.tile([C, 2], F32)
        nc.sync.dma_start(out=ar[:], in_=arout[:])
        mean = bp.tile([C, 1], F32)
        var = bp.tile([C, 1], F32)
        nc.vector.tensor_scalar_mul(mean[:], ar[:, 0:1], 1.0 / NTOT)
        nc.vector.tensor_scalar_mul(var[:], ar[:, 1:2], 1.0 / NTOT)
        m2 = bp.tile([C, 1], F32)
        nc.vector.tensor_tensor(out=m2[:], in0=mean[:], in1=mean[:], op=OP.mult)
        nc.vector.tensor_tensor(out=var[:], in0=var[:], in1=m2[:], op=OP.subtract)
        nc.vector.tensor_scalar_add(var[:], var[:], BN_EPS)
        nc.scalar.activation(out=var[:], in_=var[:], func=AF.Sqrt)
        nc.vector.reciprocal(out=var[:], in_=var[:])  # rsqrt(var+eps)
        nc.vector.tensor_tensor(out=ab[0:C, 0:1], in0=var[:], in1=gbe[:, 0:1],
                                op=OP.mult)            # a
        nc.vector.tensor_tensor(out=m2[:], in0=ab[0:C, 0:1], in1=mean[:],
                                op=OP.mult)
        nc.vector.tensor_tensor(out=ab[0:C, 1:2], in0=gbe[:, 1:2], in1=m2[:],
                                op=OP.subtract)        # b = be - a*mean
        if dup:
            nc.vector.tensor_copy(out=ab[C:2 * C, :], in_=ab[0:C, :])


_PROGRAM = None
LAST_RESULT = None


def _get_program():
    global _PROGRAM
    if _PROGRAM is None:
        _PROGRAM = _build_program()
    return _PROGRAM


def _prep_core_inputs(points1, points2, W1, W2, W3, gs, bes, b, h):
    p1 = points1[b]          # [3, N]
    p2 = points2[b]
    q = p1[:, h * QPC:(h + 1) * QPC]            # [3, QPC]

    qf = np.concatenate([2.0 * q, np.ones((1, QPC), np.float32)], axis=0)

    # gather tables double as KNN candidate tables: rows 16g+{0,1,2} carry
    # exact xyz for the band gather; group-0 row 3 carries -|c|^2 for the
    # score matmul rhs [0:4, :]
    gtab = np.zeros((128, N), np.float32)
    gtab2 = np.zeros((128, N), np.float32)
    for g in range(8):
        gtab[16 * g + 0:16 * g + 3] = p1
        gtab2[16 * g + 0:16 * g + 3] = p2
    gtab[3] = -(p1 * p1).sum(axis=0)
    gtab2[3] = -(p2 * p2).sum(axis=0)
    qraw = np.zeros((4, QPC), np.float32)
    qraw[0:3] = q
    qsqv = (q * q).sum(axis=0).reshape(NT, 128).T.astype(np.float32)

    def dup128(w):      # [64, C] -> [128, C] duplicated
        return np.concatenate([w, w], axis=0).astype(np.float32)

    selw = np.zeros((8, 128), np.float32)
    for g in range(8):
        for c3 in range(3):
            selw[g, 16 * g + c3] = 1.0

    return {
        "selw": selw,
        "qf": qf.astype(np.float32),
        "gt": gtab, "gt2": gtab2,
        "qr": qraw, "qsq": np.ascontiguousarray(qsqv),
        "w1t": np.ascontiguousarray(W1.T).astype(np.float32),
        "w2t": dup128(np.ascontiguousarray(W2.T)),
        "w3t": dup128(np.ascontiguousarray(W3.T)),
        "gb1": np.stack([gs[0], bes[0]], axis=1).astype(np.float32),
        "gb2": np.stack([gs[1], bes[1]], axis=1).astype(np.float32),
        "gb3": np.stack([gs[2], bes[2]], axis=1).astype(np.float32),
    }


def kernel(points1, points2, k, t, W1, b1, g1, be1, W2, b2, g2, be2,
           W3, b3, g3, be3):
    # b1/b2/b3 cancel inside train-mode BatchNorm; t is unused by the net.
    assert int(np.asarray(k)) == KNN
    points1 = np.asarray(points1, np.float32)
    points2 = np.asarray(points2, np.float32)
    gs = [np.asarray(g1, np.float32), np.asarray(g2, np.float32),
          np.asarray(g3, np.float32)]
    bes = [np.asarray(be1, np.float32), np.asarray(be2, np.float32),
           np.asarray(be3, np.float32)]
    Ws = [np.asarray(W1, np.float32), np.asarray(W2, np.float32),
          np.asarray(W3, np.float32)]

    in_maps = []
    for c in range(NCORES):
        b, h = divmod(c, 2)
        in_maps.append(_prep_core_inputs(points1, points2, *Ws, gs, bes, b, h))

    nc = _get_program()
    bkr = run_bass_kernel_spmd(nc, in_maps, list(range(NCORES)))
    global LAST_RESULT
    LAST_RESULT = bkr
    res = bkr.results

    out = np.zeros((B, 3, N), np.float32)
    for c in range(NCORES):
        b, h = divmod(c, 2)
        out[b, :, h * QPC:(h + 1) * QPC] = res[c]["out"]
    return out

